# revision 10
# baseline (speedup 1.0000x reference)
"""KV page-cache scatter update on 8 Trainium2 NeuronCores.

Semantics (matches the reference):
    kv_ev = interleave(new_k, new_v)          # [T, 2H, D], head axis k0,v0,k1,v1,...
    for i in range(K):
        kv_pages[t_pages[i], t_slots[i]] = kv_ev[i]
    return kv_pages

Sharding: kv_pages is split along the page axis across the 8 cores
(256 pages each).  The host partitions each core's valid tokens by
destination half (low/high page rows) and hands the core a compacted,
interleaved update block per half plus flat destination row indices
(relative to the half, pre-scaled to SPLIT-narrowed rows).

Each core's program overlaps the scatter with the bulk copy:
  1. the 33.5MB shard copy is issued as four contiguous quarter DMAs with
     64KB descriptors — half A's two quarters first on the two HWDGE
     rings, then half B's — so ring FIFO order completes half A early
  2. the output is declared as two tensors (kv_lo / kv_hi), one per half,
     so Tile's per-tensor WAW tracking lets the indirect (SWDGE) scatter
     of half A run while half B is still copying; only half B's scatter
     remains on the serial tail
  3. update rows and dest indices stage into SBUF on gpsimd under the copy
Destinations are unique (page,slot) pairs, so padding duplicates the last
valid row (identical concurrent writes are benign).
"""

import numpy as np

from concourse import bacc, bass, mybir, tile
from concourse.bass_utils import run_bass_kernel_spmd

# Problem geometry (hardcoded per contract).
P, S, HH, D = 2048, 16, 16, 128   # pages, slots/page, 2*kv_heads, head_dim
T = 2048                          # new tokens
NCORES = 8
PC = P // NCORES                  # pages per core
RC = PC * S                       # flat rows per core (4096)
RD = HH * D                       # row width in f32 (2048 = 8KB)
SPLIT = 8                         # kv_lo/kv_hi declared with RD//SPLIT wide rows
HALF = RC // 2                    # shard rows per half (2048)
TOTAL = RC * RD

_PROGRAM_CACHE: dict[tuple, object] = {}
_LAST_IN_MAPS: list | None = None  # stashed for test.py's traced re-run


def _build_program(blocks=(1, 1), repeat: int = 1):
    """Bass program: copy shard halves in->out, scatter each half's updates.

    blocks = (lo_blocks, hi_blocks): number of 128-row update blocks per
    half.  repeat>1 replicates the body inside one NEFF for slope timing."""
    lo_blocks, hi_blocks = blocks
    n_rows = 128 * (lo_blocks + hi_blocks)
    nc = bacc.Bacc("TRN2", target_bir_lowering=False, debug=False)

    kv_in = nc.dram_tensor("kv_in", [RC, RD], mybir.dt.float32, kind="ExternalInput")
    upd = nc.dram_tensor("upd", [n_rows, RD], mybir.dt.float32, kind="ExternalInput")
    dest = nc.dram_tensor("dest", [n_rows, 1], mybir.dt.int32, kind="ExternalInput")
    hrows = HALF * SPLIT
    rowd = RD // SPLIT
    kv_lo = nc.dram_tensor("kv_lo", [hrows, rowd], mybir.dt.float32, kind="ExternalOutput")
    kv_hi = nc.dram_tensor("kv_hi", [hrows, rowd], mybir.dt.float32, kind="ExternalOutput")
    halves = [kv_lo, kv_hi]
    nblocks = [lo_blocks, hi_blocks]
    q = TOTAL // 4

    with tile.TileContext(nc) as tc:
        with tc.tile_pool(name="sbuf", bufs=max(4, 2 * (lo_blocks + hi_blocks))) as pool:
            for _rep in range(repeat):
                # copy half A then half B: two contiguous quarter DMAs each
                # (64KB descriptors), one per HWDGE ring; B's descriptors sit
                # behind A's in each ring's FIFO, so half A completes early
                for h in range(2):
                    for c, eng in ((0, nc.sync), (1, nc.scalar)):
                        ap = [[8192, q // 8192], [1, 8192]]
                        eng.dma_start(out=bass.AP(halves[h], c * q, ap),
                                      in_=bass.AP(kv_in, h * (TOTAL // 2) + c * q, ap))

                # stage update rows + dest indices on gpsimd under the copy
                tiles = [[], []]
                row0 = 0
                for h in range(2):
                    for _b in range(nblocks[h]):
                        utile = pool.tile([128, RD], mybir.dt.float32)
                        dtile = pool.tile([128, 1], mybir.dt.int32)
                        nc.gpsimd.dma_start(out=dtile[:], in_=dest[row0:row0 + 128, :])
                        nc.gpsimd.dma_start(out=utile[:], in_=upd[row0:row0 + 128, :])
                        tiles[h].append((utile, dtile))
                        row0 += 128

                # scatter half A (WAW-scoped to kv_lo: overlaps half B's
                # copy), then half B after its copy lands
                for h in range(2):
                    for utile, dtile in tiles[h]:
                        nc.gpsimd.indirect_dma_start(
                            out=halves[h][:],
                            out_offset=bass.IndirectOffsetOnAxis(ap=dtile[:, :1], axis=0),
                            in_=utile[:],
                            in_offset=None,
                        )

    nc.compile()
    return nc


def kernel(kv_pages, t_pages, t_slots, new_k, new_v, K):
    kv_pages = np.asarray(kv_pages)
    t_pages = np.asarray(t_pages)
    t_slots = np.asarray(t_slots)
    new_k = np.asarray(new_k)
    new_v = np.asarray(new_v)
    k_valid = int(np.asarray(K))

    out_dtype = kv_pages.dtype
    Tn, Hn, Dn = new_k.shape

    # interleave K/V along the head axis: [T, 2H, D] -> flat [T, RD]
    kv_ev = np.empty((Tn, 2 * Hn, Dn), dtype=out_dtype)
    kv_ev[:, 0::2, :] = new_k
    kv_ev[:, 1::2, :] = new_v
    kv_ev = kv_ev.reshape(Tn, 2 * Hn * Dn)

    tp = t_pages[:k_valid].astype(np.int64)
    ts = t_slots[:k_valid].astype(np.int64)
    flat_rows = tp * S + ts            # global flat row per token
    core_of = tp // PC

    kv_flat = kv_pages.reshape(P * S, RD)

    # per (core, half) selections and block counts
    sels = {}
    blocks = [1, 1]
    for c in range(NCORES):
        base = c * RC
        local = flat_rows[core_of == c] - base
        order = np.nonzero(core_of == c)[0]
        for h in range(2):
            m = (local >= h * HALF) & (local < (h + 1) * HALF)
            sels[(c, h)] = (order[m], local[m] - h * HALF)
            blocks[h] = max(blocks[h], -(-max(1, m.sum()) // 128))
    blocks = tuple(blocks)

    if blocks not in _PROGRAM_CACHE:
        _PROGRAM_CACHE[blocks] = _build_program(blocks)
    nc = _PROGRAM_CACHE[blocks]

    lo_blocks, hi_blocks = blocks
    n_rows = 128 * (lo_blocks + hi_blocks)
    in_maps = []
    for c in range(NCORES):
        upd_c = np.empty((n_rows, RD), dtype=out_dtype)
        dest_c = np.empty((n_rows, 1), dtype=np.int32)
        row0 = 0
        for h in range(2):
            nb = (lo_blocks, hi_blocks)[h]
            sel, rel = sels[(c, h)]
            n = len(sel)
            if n > 0:
                upd_c[row0:row0 + n] = kv_ev[sel]
                dest_c[row0:row0 + n, 0] = rel * SPLIT
                upd_c[row0 + n:row0 + 128 * nb] = upd_c[row0 + n - 1]
                dest_c[row0 + n:row0 + 128 * nb, 0] = dest_c[row0 + n - 1, 0]
            else:
                # no updates in this half: rewrite its row 0 with original data
                upd_c[row0:row0 + 128 * nb] = kv_flat[c * RC + h * HALF]
                dest_c[row0:row0 + 128 * nb, 0] = 0
            row0 += 128 * nb
        in_maps.append({
            "kv_in": np.ascontiguousarray(kv_flat[c * RC:(c + 1) * RC]),
            "upd": upd_c,
            "dest": dest_c,
        })

    global _LAST_IN_MAPS
    _LAST_IN_MAPS = in_maps
    res = run_bass_kernel_spmd(nc, in_maps, core_ids=list(range(NCORES)))
    out = np.concatenate(
        [
            np.concatenate(
                [res.results[c]["kv_lo"].reshape(HALF, RD),
                 res.results[c]["kv_hi"].reshape(HALF, RD)],
                axis=0,
            ).reshape(PC, S, HH, D)
            for c in range(NCORES)
        ],
        axis=0,
    )
    return out.astype(out_dtype, copy=False)


# revision 11
# speedup vs baseline: 1.2475x; 1.2475x over previous
"""KV page-cache scatter update on 8 Trainium2 NeuronCores.

Semantics (matches the reference):
    kv_ev = interleave(new_k, new_v)          # [T, 2H, D], head axis k0,v0,k1,v1,...
    for i in range(K):
        kv_pages[t_pages[i], t_slots[i]] = kv_ev[i]
    return kv_pages

Sharding: kv_pages is split along the page axis across the 8 cores
(256 pages each).  The host partitions each core's valid tokens by
destination half (low/high page rows) and hands the core a compacted,
interleaved update block per half plus flat destination row indices
(relative to the half, pre-scaled to SPLIT-narrowed rows).

Each core's program overlaps the scatter with the bulk copy:
  1. the 33.5MB shard copy is issued as four contiguous quarter DMAs with
     64KB descriptors — half A's two quarters first on the two HWDGE
     rings, then half B's — so ring FIFO order completes half A early
  2. the output is declared as two tensors (kv_lo / kv_hi), one per half,
     so Tile's per-tensor WAW tracking lets the indirect (SWDGE) scatter
     of half A run while half B is still copying; only half B's scatter
     remains on the serial tail
  3. update rows and dest indices stage into SBUF on gpsimd under the copy
Destinations are unique (page,slot) pairs, so padding duplicates the last
valid row (identical concurrent writes are benign).
"""

import numpy as np

from concourse import bacc, bass, mybir, tile
from concourse.bass_utils import run_bass_kernel_spmd

# Problem geometry (hardcoded per contract).
P, S, HH, D = 2048, 16, 16, 128   # pages, slots/page, 2*kv_heads, head_dim
T = 2048                          # new tokens
NCORES = 8
PC = P // NCORES                  # pages per core
RC = PC * S                       # flat rows per core (4096)
RD = HH * D                       # row width in f32 (2048 = 8KB)
SPLIT = 8                         # kv_lo/kv_hi declared with RD//SPLIT wide rows
HALF = RC // 2                    # shard rows per half (2048)
TOTAL = RC * RD

_PROGRAM_CACHE: dict[tuple, object] = {}
_LAST_IN_MAPS: list | None = None  # stashed for test.py's traced re-run


def _build_program(blocks=(1, 1), repeat: int = 1):
    """Bass program: copy shard halves in->out, scatter each half's updates.

    blocks = (lo_blocks, hi_blocks): number of 128-row update blocks per
    half.  repeat>1 replicates the body inside one NEFF for slope timing."""
    lo_blocks, hi_blocks = blocks
    n_rows = 128 * (lo_blocks + hi_blocks)
    nc = bacc.Bacc("TRN2", target_bir_lowering=False, debug=False)

    kv_in = nc.dram_tensor("kv_in", [RC, RD], mybir.dt.float32, kind="ExternalInput")
    upd = nc.dram_tensor("upd", [n_rows, RD], mybir.dt.float32, kind="ExternalInput")
    dest = nc.dram_tensor("dest", [n_rows, 1], mybir.dt.int32, kind="ExternalInput")
    hrows = HALF * SPLIT
    rowd = RD // SPLIT
    kv_lo = nc.dram_tensor("kv_lo", [hrows, rowd], mybir.dt.float32, kind="ExternalOutput")
    kv_hi = nc.dram_tensor("kv_hi", [hrows, rowd], mybir.dt.float32, kind="ExternalOutput")
    halves = [kv_lo, kv_hi]
    nblocks = [lo_blocks, hi_blocks]
    q = TOTAL // 4

    # per-half copy pieces: the copy is DMA-queue-limited, so spread each
    # half over all three issuing engines (two HWDGE rings + the SWDGE ring)
    # ~37/37/25; offsets in elements, all multiples of the 8192-elem
    # (64KB-descriptor) inner run
    half_elems = TOTAL // 2
    pieces = [(0, 1638400), (1638400, 1638400), (3276800, 917504)]

    with tile.TileContext(nc) as tc:
        with tc.tile_pool(name="sbuf", bufs=max(4, 2 * (lo_blocks + hi_blocks))) as pool:
            for _rep in range(repeat):
                # copy half A across all three engines, staging in the middle
                # of gpsimd's queue, then half B; B's descriptors sit behind
                # A's in each ring's FIFO, so half A completes early
                engs = [nc.sync, nc.scalar, nc.gpsimd]
                tiles = [[], []]

                def copy_half(h):
                    for (off, sz), eng in zip(pieces, engs):
                        ap = [[8192, sz // 8192], [1, 8192]]
                        eng.dma_start(out=bass.AP(halves[h], off, ap),
                                      in_=bass.AP(kv_in, h * half_elems + off, ap))

                copy_half(0)
                # stage update rows + dest indices on gpsimd under the copy
                row0 = 0
                for h in range(2):
                    for _b in range(nblocks[h]):
                        utile = pool.tile([128, RD], mybir.dt.float32)
                        dtile = pool.tile([128, 1], mybir.dt.int32)
                        nc.gpsimd.dma_start(out=dtile[:], in_=dest[row0:row0 + 128, :])
                        nc.gpsimd.dma_start(out=utile[:], in_=upd[row0:row0 + 128, :])
                        tiles[h].append((utile, dtile))
                        row0 += 128
                copy_half(1)

                # scatter half A (WAW-scoped to kv_lo: overlaps half B's
                # copy), then half B after its copy lands
                for h in range(2):
                    for utile, dtile in tiles[h]:
                        nc.gpsimd.indirect_dma_start(
                            out=halves[h][:],
                            out_offset=bass.IndirectOffsetOnAxis(ap=dtile[:, :1], axis=0),
                            in_=utile[:],
                            in_offset=None,
                        )

    nc.compile()
    return nc


def kernel(kv_pages, t_pages, t_slots, new_k, new_v, K):
    kv_pages = np.asarray(kv_pages)
    t_pages = np.asarray(t_pages)
    t_slots = np.asarray(t_slots)
    new_k = np.asarray(new_k)
    new_v = np.asarray(new_v)
    k_valid = int(np.asarray(K))

    out_dtype = kv_pages.dtype
    Tn, Hn, Dn = new_k.shape

    # interleave K/V along the head axis: [T, 2H, D] -> flat [T, RD]
    kv_ev = np.empty((Tn, 2 * Hn, Dn), dtype=out_dtype)
    kv_ev[:, 0::2, :] = new_k
    kv_ev[:, 1::2, :] = new_v
    kv_ev = kv_ev.reshape(Tn, 2 * Hn * Dn)

    tp = t_pages[:k_valid].astype(np.int64)
    ts = t_slots[:k_valid].astype(np.int64)
    flat_rows = tp * S + ts            # global flat row per token
    core_of = tp // PC

    kv_flat = kv_pages.reshape(P * S, RD)

    # per (core, half) selections and block counts
    sels = {}
    blocks = [1, 1]
    for c in range(NCORES):
        base = c * RC
        local = flat_rows[core_of == c] - base
        order = np.nonzero(core_of == c)[0]
        for h in range(2):
            m = (local >= h * HALF) & (local < (h + 1) * HALF)
            sels[(c, h)] = (order[m], local[m] - h * HALF)
            blocks[h] = max(blocks[h], -(-max(1, m.sum()) // 128))
    blocks = tuple(blocks)

    if blocks not in _PROGRAM_CACHE:
        _PROGRAM_CACHE[blocks] = _build_program(blocks)
    nc = _PROGRAM_CACHE[blocks]

    lo_blocks, hi_blocks = blocks
    n_rows = 128 * (lo_blocks + hi_blocks)
    in_maps = []
    for c in range(NCORES):
        upd_c = np.empty((n_rows, RD), dtype=out_dtype)
        dest_c = np.empty((n_rows, 1), dtype=np.int32)
        row0 = 0
        for h in range(2):
            nb = (lo_blocks, hi_blocks)[h]
            sel, rel = sels[(c, h)]
            n = len(sel)
            if n > 0:
                upd_c[row0:row0 + n] = kv_ev[sel]
                dest_c[row0:row0 + n, 0] = rel * SPLIT
                upd_c[row0 + n:row0 + 128 * nb] = upd_c[row0 + n - 1]
                dest_c[row0 + n:row0 + 128 * nb, 0] = dest_c[row0 + n - 1, 0]
            else:
                # no updates in this half: rewrite its row 0 with original data
                upd_c[row0:row0 + 128 * nb] = kv_flat[c * RC + h * HALF]
                dest_c[row0:row0 + 128 * nb, 0] = 0
            row0 += 128 * nb
        in_maps.append({
            "kv_in": np.ascontiguousarray(kv_flat[c * RC:(c + 1) * RC]),
            "upd": upd_c,
            "dest": dest_c,
        })

    global _LAST_IN_MAPS
    _LAST_IN_MAPS = in_maps
    res = run_bass_kernel_spmd(nc, in_maps, core_ids=list(range(NCORES)))
    out = np.concatenate(
        [
            np.concatenate(
                [res.results[c]["kv_lo"].reshape(HALF, RD),
                 res.results[c]["kv_hi"].reshape(HALF, RD)],
                axis=0,
            ).reshape(PC, S, HH, D)
            for c in range(NCORES)
        ],
        axis=0,
    )
    return out.astype(out_dtype, copy=False)
